# revision 14
# baseline (speedup 1.0000x reference)
"""Trainium2 Bass kernel for nn_EntropyLoss (retrieval_knn).

Math: per (l,b) sample x = feats[l,b].reshape(C, H*W), the heavy part is the
C x C gram matrix over D = H*W = 65536.  Everything after the gram (pairwise
distances, 7th-smallest selection, per-layer sums, log, variance) touches only
C*C = 4096 values per sample and runs on host, replicating the fp32 reference
arithmetic.

Device layout (data-parallel over the 24 = L*B samples, 3 per core):
  - Samples 0,1 are processed as a pair: DMA slabs [128, F] with sample 0 on
    partitions 0:64 and sample 1 on 64:128 (adjacent rows in DRAM -> one
    full-bandwidth 128-partition transfer).
  - Sample 2 is pre-reshaped on host to [128, D/2] (its two column halves
    stacked on rows), so its slabs are also plain full-bandwidth 2D loads;
    its two accumulated half-grams are summed on host.
  - PE transposes [64,128] -> [128,64] into PSUM (the PE contracts along
    partitions, so both matmul operands need d on partitions), 8 transposed
    chunks per PSUM bank [128, 512].
  - DVE/ACT alternate copying full banks PSUM -> SBUF (DMA cannot read PSUM).
  - Per chunk one fp32 matmul [K=128, M=64, N=64]; low/high-half chunks
    accumulate into partitions 0:64 / 64:128 of a [128, 64] PSUM tile.  The
    two targets sit in PE column-groups 0 and 64, so adjacent matmuls run
    concurrently on different quadrant columns.
"""

from collections import deque

import numpy as np

C = 64            # channels (gram is C x C)
PAIR = 128        # contraction chunk per matmul (PE partition limit)
GROUP_PAIRS = 4   # chunk-pairs per PSUM staging bank -> [128, 512] fp32
SLAB_F = 4096     # free columns per DMA slab ([128, 4096] tile, 2 MiB)
PIPE_DELAY = 2    # bank-groups between transpose emission and MM consumption

N_CORES = 8
L, B, HW = 3, 8, 65536
SAMPLES = L * B
S_PER_CORE = SAMPLES // N_CORES


def build_kernel(n_samples: int, D: int, repeat: int = 1, tpsum_bufs: int = 5,
                 stage_bufs: int = 3, xt_bufs: int = 5, pipe_delay: int = PIPE_DELAY):
    from concourse import bacc
    import concourse.mybir as mybir
    import concourse.tile as tile

    assert n_samples in (2, 3)
    fp32 = mybir.dt.float32
    nc = bacc.Bacc("TRN2", target_bir_lowering=False, debug=False)

    n_out = 1 if n_samples == 2 else 2
    xs = nc.dram_tensor("xs", [2, C, D], fp32, kind="ExternalInput")
    if n_samples == 3:
        xsolo = nc.dram_tensor("xsolo", [2 * C, D // 2], fp32, kind="ExternalInput")
    g2 = nc.dram_tensor("g2", [n_out, 2 * C, C], fp32, kind="ExternalOutput")

    ident_np = np.eye(2 * C, dtype=np.float32)
    ident = nc.inline_tensor(ident_np, name="ident128")

    bank_cols = 2 * GROUP_PAIRS * C  # 512
    groups_per_slab = SLAB_F // (PAIR * GROUP_PAIRS)  # 8

    with tile.TileContext(nc) as tc:
        with (
            tc.tile_pool(name="consts", bufs=1) as const_pool,
            tc.tile_pool(name="stage", bufs=stage_bufs) as stage_pool,
            tc.tile_pool(name="tpsum", bufs=tpsum_bufs, space="PSUM") as tpsum_pool,
            tc.tile_pool(name="xT", bufs=xt_bufs) as xT_pool,
            tc.tile_pool(name="gpsum", bufs=2, space="PSUM") as gpsum_pool,
            tc.tile_pool(name="outs", bufs=2) as out_pool,
        ):
            id_sb = const_pool.tile_from(ident[:])  # [128, 128]

            def run_phase(out_slot, slab_srcs):
                """slab_srcs: list of [128, SLAB_F] APs; low/high partition
                halves accumulate into partitions 0:64 / 64:128 of g_ps."""
                n_slabs = len(slab_srcs)
                mm_per_half = n_slabs * SLAB_F // PAIR
                g_ps = gpsum_pool.tile([2 * C, C], fp32)
                mm_count = [0, 0]
                pending = deque()

                def emit_mms(xT):
                    for k in range(2 * GROUP_PAIRS):
                        h = k % 2
                        cnt = mm_count[h]
                        tile_ap = xT[:, k * C:(k + 1) * C]
                        nc.tensor.matmul(
                            out=g_ps[h * C:(h + 1) * C, :],
                            lhsT=tile_ap,
                            rhs=tile_ap,
                            start=(cnt == 0),
                            stop=(cnt == mm_per_half - 1),
                            skip_group_check=True,
                        )
                        mm_count[h] += 1

                for src in slab_srcs:
                    stage = stage_pool.tile([2 * C, SLAB_F], fp32)
                    nc.sync.dma_start(stage[:], src)
                    for b in range(groups_per_slab):
                        ps = tpsum_pool.tile([PAIR, bank_cols], fp32)
                        for j in range(GROUP_PAIRS):
                            jj = b * GROUP_PAIRS + j
                            # [128,128] full-partition transpose: output cols
                            # 0:64 = low half's chunk, 64:128 = high half's.
                            nc.tensor.transpose(
                                ps[:, j * PAIR:(j + 1) * PAIR],
                                stage[:, jj * PAIR:(jj + 1) * PAIR],
                                id_sb[:],
                            )
                        xT = xT_pool.tile([PAIR, bank_cols], fp32)
                        if b % 2 == 0:
                            nc.vector.tensor_copy(xT, ps)
                        else:
                            nc.scalar.copy(xT, ps)
                        pending.append(xT)
                        if len(pending) > pipe_delay:
                            emit_mms(pending.popleft())
                while pending:
                    emit_mms(pending.popleft())

                g2_sb = out_pool.tile([2 * C, C], fp32)
                nc.vector.tensor_copy(g2_sb, g_ps)
                nc.sync.dma_start(g2[out_slot], g2_sb)

            # Phase 1: samples 0 and 1 stacked on partitions, one slab per
            # SLAB_F columns.  Output slot 0 = [gram(s0); gram(s1)].
            pair_srcs = [
                xs[0:2, :, w * SLAB_F:(w + 1) * SLAB_F].rearrange(
                    "s c f -> (s c) f"
                )
                for w in range(D // SLAB_F)
            ]
            solo_srcs = [
                xsolo[:, u * SLAB_F:(u + 1) * SLAB_F]
                for u in range(D // 2 // SLAB_F)
            ] if n_samples == 3 else None

            # repeat > 1 re-runs the whole computation (benchmarking only;
            # outputs are simply rewritten).
            for _ in range(repeat):
                run_phase(0, pair_srcs)
                # Phase 2: sample 2 (host-restacked to [128, D/2]).  Output
                # slot 1 = [half_gram_A; half_gram_B], summed on host.
                if n_samples == 3:
                    run_phase(1, solo_srcs)

    nc.compile()
    return nc


_KERNEL_CACHE = {}


def _get_kernel(n_samples: int, D: int):
    key = (n_samples, D)
    if key not in _KERNEL_CACHE:
        _KERNEL_CACHE[key] = build_kernel(n_samples, D)
    return _KERNEL_CACHE[key]


def grams_from_g2(g2: np.ndarray, n_cores: int = N_CORES) -> np.ndarray:
    """g2: [n_cores, 2, 128, 64] -> grams [3 * n_cores, 64, 64]."""
    grams = np.zeros((3 * n_cores, C, C), dtype=np.float32)
    for i in range(n_cores):
        grams[3 * i + 0] = g2[i, 0, :C, :]
        grams[3 * i + 1] = g2[i, 0, C:, :]
        grams[3 * i + 2] = g2[i, 1, :C, :] + g2[i, 1, C:, :]
    return grams


def _postprocess(grams: np.ndarray):
    """grams: [SAMPLES, C, C] fp32 -> scalar, replicating reference fp32 math."""
    K = C // 10
    rballs = np.zeros((SAMPLES, C), dtype=np.float32)
    for i in range(SAMPLES):
        g = grams[i]
        sq = np.diagonal(g).copy()
        d2 = (sq[:, None] + sq[None, :]) - np.float32(2.0) * g
        d2 = np.clip(d2, np.float32(1e-8), None)
        dist = np.sqrt(d2, dtype=np.float32)
        rballs[i] = np.sort(dist, axis=-1)[:, K]

    rb = rballs.reshape(L, B * C)
    try:
        import jax

        cpu = jax.devices("cpu")[0]
        with jax.default_device(cpu):
            import jax.numpy as jnp

            H = jnp.sum(jnp.asarray(rb), axis=-1)
            ent = jnp.log(H + 1.0)
            delta = ent[1:] - ent[:-1]
            var = jnp.var(delta, ddof=1)
            return np.asarray(var, dtype=np.float32)
    except Exception:
        H = rb.astype(np.float32).sum(axis=-1)
        ent = np.log(H + np.float32(1.0)).astype(np.float32)
        delta = ent[1:] - ent[:-1]
        n = delta.shape[0]
        mean = np.float32(delta.mean())
        var = np.float32(((delta - mean) ** 2).sum() / np.float32(n - 1))
        return np.asarray(var, dtype=np.float32)


def kernel(feats: np.ndarray) -> np.ndarray:
    from concourse.bass_utils import run_bass_kernel_spmd

    feats = np.ascontiguousarray(feats, dtype=np.float32)
    x = feats.reshape(SAMPLES, C, HW)

    nc = _get_kernel(S_PER_CORE, HW)
    in_maps = []
    for i in range(N_CORES):
        s0 = i * S_PER_CORE
        solo = np.concatenate(
            [x[s0 + 2, :, : HW // 2], x[s0 + 2, :, HW // 2:]], axis=0
        )
        in_maps.append({"xs": x[s0:s0 + 2], "xsolo": solo})
    res = run_bass_kernel_spmd(nc, in_maps, core_ids=list(range(N_CORES)))
    g2 = np.stack([r["g2"] for r in res.results], axis=0)  # [8, 2, 128, 64]
    grams = grams_from_g2(g2)
    return _postprocess(grams)


if __name__ == "__main__":
    feats = np.random.default_rng(0).standard_normal(
        (L, B, C, 256, 256)
    ).astype(np.float32)
    print(kernel(feats))


# revision 19
# speedup vs baseline: 2.0823x; 2.0823x over previous
"""Trainium2 Bass kernel for nn_EntropyLoss (retrieval_knn).

Math: per (l,b) sample x = feats[l,b].reshape(C, H*W), the heavy part is the
C x C gram matrix over D = H*W = 65536.  Everything after the gram (pairwise
distances, 7th-smallest selection, per-layer sums, log, variance) touches only
C*C = 4096 values per sample and runs on host, replicating the fp32 reference
arithmetic.

Active design (`build_kernel_mm` + `pack_xt`, data-parallel, 3 samples/core):
  - The PE contracts along partitions, so both matmul operands need d on
    partitions.  Instead of transposing on device, the host pre-tiles each
    sample into its transposed SBUF image: slabs [128, 4096] where column
    w*64 + c of partition p holds x[c, g*8192 + w*128 + p].  Every 64-column
    slice is a ready matmul operand [K=128, M=N=64], and each slab is one
    fully-contiguous full-bandwidth 2 MiB DMA.
  - Per chunk one fp32 matmul; chunks alternate (w % 2) between partition
    halves 0:64 / 64:128 of a [128, 64] gram PSUM tile.  The two targets sit
    in PE column-groups 0 and 64, so adjacent matmuls run concurrently on
    different quadrant columns.  Host folds the two half-grams.
  - This leaves the PE stream as the only nontrivial device work (~82 us of
    matmul at the fp32 roofline per core); DVE/ACT/PSUM staging all idle.

`build_kernel` (v1) is the earlier all-on-device variant that transposes via
the PE and stages through PSUM; kept for reference/fallback (~2.5x slower).
"""

from collections import deque

import numpy as np

C = 64            # channels (gram is C x C)
PAIR = 128        # contraction chunk per matmul (PE partition limit)
GROUP_PAIRS = 4   # chunk-pairs per PSUM staging bank -> [128, 512] fp32
SLAB_F = 4096     # free columns per DMA slab ([128, 4096] tile, 2 MiB)
PIPE_DELAY = 2    # bank-groups between transpose emission and MM consumption

N_CORES = 8
L, B, HW = 3, 8, 65536
SAMPLES = L * B
S_PER_CORE = SAMPLES // N_CORES


def build_kernel(n_samples: int, D: int, repeat: int = 1, tpsum_bufs: int = 5,
                 stage_bufs: int = 3, xt_bufs: int = 5, pipe_delay: int = PIPE_DELAY):
    from concourse import bacc
    import concourse.mybir as mybir
    import concourse.tile as tile

    assert n_samples in (2, 3)
    fp32 = mybir.dt.float32
    nc = bacc.Bacc("TRN2", target_bir_lowering=False, debug=False)

    n_out = 1 if n_samples == 2 else 2
    xs = nc.dram_tensor("xs", [2, C, D], fp32, kind="ExternalInput")
    if n_samples == 3:
        xsolo = nc.dram_tensor("xsolo", [2 * C, D // 2], fp32, kind="ExternalInput")
    g2 = nc.dram_tensor("g2", [n_out, 2 * C, C], fp32, kind="ExternalOutput")

    ident_np = np.eye(2 * C, dtype=np.float32)
    ident = nc.inline_tensor(ident_np, name="ident128")

    bank_cols = 2 * GROUP_PAIRS * C  # 512
    groups_per_slab = SLAB_F // (PAIR * GROUP_PAIRS)  # 8

    with tile.TileContext(nc) as tc:
        with (
            tc.tile_pool(name="consts", bufs=1) as const_pool,
            tc.tile_pool(name="stage", bufs=stage_bufs) as stage_pool,
            tc.tile_pool(name="tpsum", bufs=tpsum_bufs, space="PSUM") as tpsum_pool,
            tc.tile_pool(name="xT", bufs=xt_bufs) as xT_pool,
            tc.tile_pool(name="gpsum", bufs=2, space="PSUM") as gpsum_pool,
            tc.tile_pool(name="outs", bufs=2) as out_pool,
        ):
            id_sb = const_pool.tile_from(ident[:])  # [128, 128]

            def run_phase(out_slot, slab_srcs):
                """slab_srcs: list of [128, SLAB_F] APs; low/high partition
                halves accumulate into partitions 0:64 / 64:128 of g_ps."""
                n_slabs = len(slab_srcs)
                mm_per_half = n_slabs * SLAB_F // PAIR
                g_ps = gpsum_pool.tile([2 * C, C], fp32)
                mm_count = [0, 0]
                pending = deque()

                def emit_mms(xT):
                    for k in range(2 * GROUP_PAIRS):
                        h = k % 2
                        cnt = mm_count[h]
                        tile_ap = xT[:, k * C:(k + 1) * C]
                        nc.tensor.matmul(
                            out=g_ps[h * C:(h + 1) * C, :],
                            lhsT=tile_ap,
                            rhs=tile_ap,
                            start=(cnt == 0),
                            stop=(cnt == mm_per_half - 1),
                            skip_group_check=True,
                        )
                        mm_count[h] += 1

                for src in slab_srcs:
                    stage = stage_pool.tile([2 * C, SLAB_F], fp32)
                    nc.sync.dma_start(stage[:], src)
                    for b in range(groups_per_slab):
                        ps = tpsum_pool.tile([PAIR, bank_cols], fp32)
                        for j in range(GROUP_PAIRS):
                            jj = b * GROUP_PAIRS + j
                            # [128,128] full-partition transpose: output cols
                            # 0:64 = low half's chunk, 64:128 = high half's.
                            nc.tensor.transpose(
                                ps[:, j * PAIR:(j + 1) * PAIR],
                                stage[:, jj * PAIR:(jj + 1) * PAIR],
                                id_sb[:],
                            )
                        xT = xT_pool.tile([PAIR, bank_cols], fp32)
                        if b % 2 == 0:
                            nc.vector.tensor_copy(xT, ps)
                        else:
                            nc.scalar.copy(xT, ps)
                        pending.append(xT)
                        if len(pending) > pipe_delay:
                            emit_mms(pending.popleft())
                while pending:
                    emit_mms(pending.popleft())

                g2_sb = out_pool.tile([2 * C, C], fp32)
                nc.vector.tensor_copy(g2_sb, g_ps)
                nc.sync.dma_start(g2[out_slot], g2_sb)

            # Phase 1: samples 0 and 1 stacked on partitions, one slab per
            # SLAB_F columns.  Output slot 0 = [gram(s0); gram(s1)].
            pair_srcs = [
                xs[0:2, :, w * SLAB_F:(w + 1) * SLAB_F].rearrange(
                    "s c f -> (s c) f"
                )
                for w in range(D // SLAB_F)
            ]
            solo_srcs = [
                xsolo[:, u * SLAB_F:(u + 1) * SLAB_F]
                for u in range(D // 2 // SLAB_F)
            ] if n_samples == 3 else None

            # repeat > 1 re-runs the whole computation (benchmarking only;
            # outputs are simply rewritten).
            for _ in range(repeat):
                run_phase(0, pair_srcs)
                # Phase 2: sample 2 (host-restacked to [128, D/2]).  Output
                # slot 1 = [half_gram_A; half_gram_B], summed on host.
                if n_samples == 3:
                    run_phase(1, solo_srcs)

    nc.compile()
    return nc


def build_kernel_mm(n_samples: int, D: int, repeat: int = 1,
                    slab_bufs: int = 4, slab_f: int = SLAB_F):
    """Pure-matmul kernel: host supplies pre-tiled transposed data.

    Input xt: [n_samples, n_slabs, 128, SLAB_F] where slab g, partition p,
    column w*C + c holds x[c, g*(SLAB_F*2) ... ]: d = g*8*PAIR... precisely
    xt[s, g, p, w*C + c] = x[s, c, g*(SLAB_F//C*PAIR//2)... see host packing
    in `pack_xt`: d = g*8192 + w*128 + p.  Each [128, SLAB_F] slab is one
    full-bandwidth contiguous DMA; every C-column slice is a ready matmul
    operand [K=128, 64].  Chunks alternate (w % 2) between partition halves
    0:64 / 64:128 of the gram PSUM tile (PE column groups 0/64 run
    concurrently); host folds the two half-grams.
    """
    from concourse import bacc
    import concourse.mybir as mybir
    import concourse.tile as tile

    fp32 = mybir.dt.float32
    nc = bacc.Bacc("TRN2", target_bir_lowering=False, debug=False)

    chunks_per_slab = slab_f // C
    n_slabs = D // (chunks_per_slab * PAIR)
    xt = nc.dram_tensor(
        "xt", [n_samples, n_slabs, PAIR, slab_f], fp32, kind="ExternalInput"
    )
    g2 = nc.dram_tensor("g2", [n_samples, 2 * C, C], fp32, kind="ExternalOutput")

    with tile.TileContext(nc) as tc:
        with (
            tc.tile_pool(name="slab", bufs=slab_bufs) as slab_pool,
            tc.tile_pool(name="gpsum", bufs=2, space="PSUM") as gpsum_pool,
            tc.tile_pool(name="outs", bufs=2) as out_pool,
        ):
            for _ in range(repeat):
                for s in range(n_samples):
                    g_ps = gpsum_pool.tile([2 * C, C], fp32)
                    mm_count = [0, 0]
                    mm_per_half = n_slabs * chunks_per_slab // 2
                    for g in range(n_slabs):
                        slab = slab_pool.tile([PAIR, slab_f], fp32)
                        nc.sync.dma_start(slab[:], xt[s, g])
                        for w in range(chunks_per_slab):
                            h = w % 2
                            cnt = mm_count[h]
                            tap = slab[:, w * C:(w + 1) * C]
                            nc.tensor.matmul(
                                out=g_ps[h * C:(h + 1) * C, :],
                                lhsT=tap,
                                rhs=tap,
                                start=(cnt == 0),
                                stop=(cnt == mm_per_half - 1),
                                skip_group_check=True,
                            )
                            mm_count[h] += 1
                    g2_sb = out_pool.tile([2 * C, C], fp32)
                    nc.vector.tensor_copy(g2_sb, g_ps)
                    nc.sync.dma_start(g2[s], g2_sb)

    nc.compile()
    return nc


def pack_xt(x: np.ndarray, slab_f: int = SLAB_F) -> np.ndarray:
    """x: [n_samples, C, D] -> [n_samples, n_slabs, 128, slab_f] pre-tiled
    transposed layout: xt[s, g, p, w*C + c] = x[s, c, g*(2*slab_f) + w*128 + p]."""
    ns, c, d = x.shape
    per_slab_d = slab_f // C * PAIR
    n_slabs = d // per_slab_d
    v = x.reshape(ns, c, n_slabs, slab_f // C, PAIR)  # (s, c, g, w, p)
    return np.ascontiguousarray(v.transpose(0, 2, 4, 3, 1)).reshape(
        ns, n_slabs, PAIR, slab_f
    )


_KERNEL_CACHE = {}


def _get_kernel(n_samples: int, D: int):
    key = (n_samples, D)
    if key not in _KERNEL_CACHE:
        _KERNEL_CACHE[key] = build_kernel_mm(n_samples, D)
    return _KERNEL_CACHE[key]


def grams_from_g2(g2: np.ndarray, n_cores: int = N_CORES) -> np.ndarray:
    """g2 (v1 layout): [n_cores, 2, 128, 64] -> grams [3*n_cores, 64, 64]."""
    grams = np.zeros((3 * n_cores, C, C), dtype=np.float32)
    for i in range(n_cores):
        grams[3 * i + 0] = g2[i, 0, :C, :]
        grams[3 * i + 1] = g2[i, 0, C:, :]
        grams[3 * i + 2] = g2[i, 1, :C, :] + g2[i, 1, C:, :]
    return grams


def grams_from_g2_mm(g2: np.ndarray) -> np.ndarray:
    """g2 (v2 layout): [n_total_samples, 128, 64] half-gram pairs."""
    return (g2[:, :C, :] + g2[:, C:, :]).astype(np.float32)


def _postprocess(grams: np.ndarray):
    """grams: [SAMPLES, C, C] fp32 -> scalar, replicating reference fp32 math."""
    K = C // 10
    rballs = np.zeros((SAMPLES, C), dtype=np.float32)
    for i in range(SAMPLES):
        g = grams[i]
        sq = np.diagonal(g).copy()
        d2 = (sq[:, None] + sq[None, :]) - np.float32(2.0) * g
        d2 = np.clip(d2, np.float32(1e-8), None)
        dist = np.sqrt(d2, dtype=np.float32)
        rballs[i] = np.sort(dist, axis=-1)[:, K]

    rb = rballs.reshape(L, B * C)
    try:
        import jax

        cpu = jax.devices("cpu")[0]
        with jax.default_device(cpu):
            import jax.numpy as jnp

            H = jnp.sum(jnp.asarray(rb), axis=-1)
            ent = jnp.log(H + 1.0)
            delta = ent[1:] - ent[:-1]
            var = jnp.var(delta, ddof=1)
            return np.asarray(var, dtype=np.float32)
    except Exception:
        H = rb.astype(np.float32).sum(axis=-1)
        ent = np.log(H + np.float32(1.0)).astype(np.float32)
        delta = ent[1:] - ent[:-1]
        n = delta.shape[0]
        mean = np.float32(delta.mean())
        var = np.float32(((delta - mean) ** 2).sum() / np.float32(n - 1))
        return np.asarray(var, dtype=np.float32)


def kernel(feats: np.ndarray) -> np.ndarray:
    from concourse.bass_utils import run_bass_kernel_spmd

    feats = np.ascontiguousarray(feats, dtype=np.float32)
    x = feats.reshape(SAMPLES, C, HW)

    nc = _get_kernel(S_PER_CORE, HW)
    in_maps = [
        {"xt": pack_xt(x[i * S_PER_CORE:(i + 1) * S_PER_CORE])}
        for i in range(N_CORES)
    ]
    res = run_bass_kernel_spmd(nc, in_maps, core_ids=list(range(N_CORES)))
    g2 = np.concatenate([r["g2"] for r in res.results], axis=0)  # [24,128,64]
    grams = grams_from_g2_mm(g2)
    return _postprocess(grams)


if __name__ == "__main__":
    feats = np.random.default_rng(0).standard_normal(
        (L, B, C, 256, 256)
    ).astype(np.float32)
    print(kernel(feats))
